# revision 27
# baseline (speedup 1.0000x reference)
"""AdvancedMuonAttention Trainium2 kernel (8 NeuronCores, SPMD, no collectives).

Sharding: core c -> (batch b = c//2, query half q = c%2).  Each core computes
its [1024, 1024] slice of the output (including RMSNorm) entirely locally:
q-projection on its 1024 query rows, k/v-projections on the full 2048 keys of
its batch (duplicated across the 2 cores sharing a batch), attention, output
projection, RMSNorm.  The host shards inputs / reassembles outputs.

Device-side layout choices (validated by probes):
  - activations channels-first [D, S]; weights pre-transposed [D_in, D_out]
  - fp32r (fp32 rounded to 11 mantissa bits, full PE speed) for projections
  - bf16 for qn/kn/P/mask/v (2x DVE modes); fp32 PSUM accumulation
  - scoresT [k, q] orientation: softmax sums ride the ctx matmul via a ones
    column appended to v (M=65); division by sums is applied to ctx (64
    values per (head, q)) instead of P (2048 values)
  - exp without max subtraction (scores are bounded, rows can't be all-masked)
  - masking = multiply exp(scores) by {0,1} mask (identical to -1e9 select)
"""
import sys
import numpy as np
import ml_dtypes

sys.path.insert(0, "/opt/trn_rl_repo")

import concourse.bacc as bacc
import concourse.mybir as mybir
import concourse.tile as tile
from concourse.bass_utils import run_bass_kernel_spmd

F32 = mybir.dt.float32
F32R = mybir.dt.float32r
BF16 = mybir.dt.bfloat16

B, S, D, H, DK = 4, 2048, 1024, 16, 64
SQ = 1024            # query rows per core
P = 128              # partitions
NCORES = 8
NKT = S // P         # 16 k-tiles
NJ = H // 2          # 8 head pairs / d-block pairs
EPS = 1e-8


def _f32r_round(x):
    """RNE-round fp32 to 11 mantissa bits (the PE's fp32r operand format)."""
    u = np.ascontiguousarray(x, dtype=np.float32).view(np.uint32)
    r = ((u.astype(np.uint64) + 0x7FF + ((u >> 12) & 1)) & 0xFFFFF000).astype(np.uint32)
    return r.view(np.float32)


def build_nc(debug=False):
    nc = bacc.Bacc("TRN2", target_bir_lowering=False)

    # inputs ----------------------------------------------------------------
    qt_in = nc.declare_dram_parameter("qt_in", [D, SQ], BF16, isOutput=False)
    kt_in = nc.declare_dram_parameter("kt_in", [D, S], F32R, isOutput=False)
    vt_in = nc.declare_dram_parameter("vt_in", [D, S], BF16, isOutput=False)
    maskt = nc.declare_dram_parameter("maskt", [S, SQ], BF16, isOutput=False)
    wqt = nc.declare_dram_parameter("wqt", [D, D], BF16, isOutput=False)
    wkt = nc.declare_dram_parameter("wkt", [D, D], F32R, isOutput=False)
    wvt = nc.declare_dram_parameter("wvt", [D, D], BF16, isOutput=False)
    wot = nc.declare_dram_parameter("wot", [D, D], F32R, isOutput=False)
    bqt = nc.declare_dram_parameter("bqt", [P, 8], F32, isOutput=False)
    bkt = nc.declare_dram_parameter("bkt", [P, 8], F32, isOutput=False)
    bvr = nc.declare_dram_parameter("bvr", [1, D], F32R, isOutput=False)
    bor = nc.declare_dram_parameter("bor", [1, D], F32R, isOutput=False)
    naqw = nc.declare_dram_parameter("naqw", [P, P], F32R, isOutput=False)
    nakw = nc.declare_dram_parameter("nakw", [P, P], F32R, isOutput=False)
    naqb = nc.declare_dram_parameter("naqb", [P, 1], F32, isOutput=False)
    nakb = nc.declare_dram_parameter("nakb", [P, 1], F32, isOutput=False)
    tscp = nc.declare_dram_parameter("tscp", [P, 8], F32, isOutput=False)
    rmsw = nc.declare_dram_parameter("rmsw", [P, D], F32, isOutput=False)
    onesr = nc.declare_dram_parameter("onesr", [1, P], F32R, isOutput=False)
    out = nc.declare_dram_parameter("out", [SQ, D], F32, isOutput=True)

    sums_d = nc.dram_tensor("sums_d", [H, SQ], F32)
    recip_d = nc.dram_tensor("recip_d", [H, SQ], F32)
    if debug:
        qnd = nc.declare_dram_parameter("qnd", [NJ * P, SQ], BF16, isOutput=True)
        knd = nc.declare_dram_parameter("knd", [NJ * P, S], BF16, isOutput=True)
        vd = nc.declare_dram_parameter("vd", [NKT * P, H * (DK + 1)], BF16, isOutput=True)
        ctxd = nc.declare_dram_parameter("ctxd", [NJ * P, SQ], F32, isOutput=True)
        sumsd = nc.declare_dram_parameter("sumsd", [H, SQ], F32, isOutput=True)
        ctxrd = nc.declare_dram_parameter("ctxrd", [NJ * P, SQ], F32, isOutput=True)

    AF = mybir.ActivationFunctionType
    OP = mybir.AluOpType

    with tile.TileContext(nc) as tc:
        import contextlib
        es = contextlib.ExitStack()
        with es:
            # long-lived pools (alloc'd bottom of SBUF stack, freed at end)
            const = es.enter_context(tc.tile_pool(name="const", bufs=1))
            qcp = es.enter_context(tc.tile_pool(name="qcp", bufs=9))
            wrk = es.enter_context(tc.tile_pool(name="wrk", bufs=1))
            es2 = es.enter_context(contextlib.ExitStack())
            knp = es2.enter_context(tc.tile_pool(name="knp", bufs=1))
            vap = es2.enter_context(tc.tile_pool(name="vap", bufs=1))

            # constants
            onesr_sb = const.tile([1, P], F32R, name="onesr_sb")
            nc.sync.dma_start(out=onesr_sb[:, :], in_=onesr[:, :])
            bvr_sb = const.tile([1, D], F32R, name="bvr_sb")
            nc.sync.dma_start(out=bvr_sb[:, :], in_=bvr[:, :])
            bor_sb = const.tile([1, D], F32R, name="bor_sb")
            nc.sync.dma_start(out=bor_sb[:, :], in_=bor[:, :])

            # long-lived tensors
            knt = [knp.tile([P, S], BF16, tag=f"kn{j}", name=f"knt{j}")
                   for j in range(NJ)]
            vaug = [vap.tile([P, H, DK + 1], BF16, tag=f"v{st}", name=f"vaug{st}")
                    for st in range(NKT)]
            qn = [qcp.tile([P, SQ], BF16, tag="qc", name=f"qn{j}")
                  for j in range(NJ)]

            # shared projection psum pools (V/K/Q) so phases can overlap
            esp = es.enter_context(contextlib.ExitStack())
            prj = esp.enter_context(tc.tile_pool(name="prj", bufs=4, space="PSUM"))
            prn = esp.enter_context(tc.tile_pool(name="prn", bufs=2, space="PSUM"))
            inp = esp.enter_context(tc.tile_pool(name="inp", bufs=2))

            # ---------------- phase V: v = V @ wv.T + bv (all bf16) --------
            wvkp = esp.enter_context(contextlib.ExitStack())
            wvp = wvkp.enter_context(tc.tile_pool(name="wvp", bufs=1))
            wkp = wvkp.enter_context(tc.tile_pool(name="wkp", bufs=1))
            if True:
                wv_t = []
                for ib in range(8):
                    w_t = wvp.tile([P, D], BF16, tag=f"wv{ib}", name=f"wv{ib}")
                    nc.sync.dma_start(out=w_t[:, :], in_=wvt[ib * P:(ib + 1) * P, :])
                    wv_t.append(w_t)
                for st in range(NKT):
                    nc.vector.memset(vaug[st][:, :, DK:DK + 1], 1.0)
                bqt_sb = const.tile([P, 8], F32, name="bqt_sb")
                nc.sync.dma_start(out=bqt_sb[:, :], in_=bqt[:, :])
                bkt_sb = const.tile([P, 8], F32, name="bkt_sb")
                nc.sync.dma_start(out=bkt_sb[:, :], in_=bkt[:, :])
                naqw_sb = const.tile([P, P], F32R, name="naqw_sb")
                nc.sync.dma_start(out=naqw_sb[:, :], in_=naqw[:, :])
                nakw_sb = const.tile([P, P], F32R, name="nakw_sb")
                nc.sync.dma_start(out=nakw_sb[:, :], in_=nakw[:, :])
                naqb_sb = const.tile([P, 1], F32, name="naqb_sb")
                nc.sync.dma_start(out=naqb_sb[:, :], in_=naqb[:, :])
                nakb_sb = const.tile([P, 1], F32, name="nakb_sb")
                nc.sync.dma_start(out=nakb_sb[:, :], in_=nakb[:, :])
                tscp_sb = const.tile([P, 8], F32, name="tscp_sb")
                nc.sync.dma_start(out=tscp_sb[:, :], in_=tscp[:, :])
                for sbi in range(4):
                    vin_t = []
                    for ib in range(8):
                        t = inp.tile([P, 512], BF16, tag=f"in{ib}", name=f"vin{ib}_{sbi}")
                        nc.sync.dma_start(
                            out=t[:, :],
                            in_=vt_in[ib * P:(ib + 1) * P, sbi * 512:(sbi + 1) * 512])
                        vin_t.append(t)
                    for str_ in range(4):
                        st = sbi * 4 + str_
                        for ob in range(2):
                            pv = prj.tile([P, 512], F32, tag="prj", name=f"pv{st}_{ob}")
                            for ib in range(8):
                                nc.tensor.matmul(
                                    pv[:, :],
                                    lhsT=vin_t[ib][:, str_ * P:(str_ + 1) * P],
                                    rhs=wv_t[ib][:, ob * 512:(ob + 1) * 512],
                                    start=(ib == 0), stop=False)
                            nc.tensor.matmul(
                                pv[:, :], lhsT=onesr_sb[:, :],
                                rhs=bvr_sb[:, ob * 512:(ob + 1) * 512],
                                start=False, stop=True)
                            nc.vector.tensor_scalar(
                                vaug[st][:, ob * 8:(ob + 1) * 8, 0:DK],
                                pv[:, :].rearrange("p (a b) -> p a b", a=8),
                                1.0, None, op0=OP.mult)

            # ---------------- phase K: kn = tanh(naK @ (wk @ K.T + bk)) ----
            with tc.tile_pool(name="ktmp", bufs=3) as ktmpp:
                wk_t = []
                for ib in range(8):
                    w_t = wkp.tile([P, D], F32R, tag=f"wk{ib}", name=f"wk{ib}")
                    nc.sync.dma_start(out=w_t[:, :], in_=wkt[ib * P:(ib + 1) * P, :])
                    wk_t.append(w_t)
                for sbi in range(4):
                    kin_t = []
                    for ib in range(8):
                        t = inp.tile([P, 512], F32R, tag=f"in{ib}", name=f"kin{ib}_{sbi}")
                        nc.sync.dma_start(
                            out=t[:, :],
                            in_=kt_in[ib * P:(ib + 1) * P, sbi * 512:(sbi + 1) * 512])
                        kin_t.append(t)
                    for j in range(NJ):
                        pk = prj.tile([P, 512], F32, tag="prj", name=f"pk{sbi}_{j}")
                        for ib in range(8):
                            nc.tensor.matmul(
                                pk[:, :],
                                lhsT=wk_t[ib][:, j * P:(j + 1) * P],
                                rhs=kin_t[ib][:, :],
                                start=(ib == 0), stop=(ib == 7))
                        kt_t = ktmpp.tile([P, 512], F32R, tag="ktmp", name=f"kt{sbi}_{j}")
                        nc.scalar.activation(kt_t[:, :], pk[:, :], AF.Identity,
                                             bias=bkt_sb[:, j:j + 1])
                        pkn = prn.tile([P, 512], F32, tag="prn", name=f"pkn{sbi}_{j}")
                        nc.tensor.matmul(pkn[:, :], lhsT=nakw_sb[:, :],
                                         rhs=kt_t[:, :], start=True, stop=True)
                        nc.scalar.activation(
                            knt[j][:, sbi * 512:(sbi + 1) * 512], pkn[:, :],
                            AF.Tanh, bias=nakb_sb[:, :])

            wvkp.close()   # free wv/wk weight pools

            # ---------------- phase Q: qn (like K, on SQ rows) + temp scale -
            with tc.tile_pool(name="wqp", bufs=1) as wqp, \
                 tc.tile_pool(name="qtmp", bufs=3) as qtmpp:
                wq_t = []
                for ib in range(8):
                    w_t = wqp.tile([P, D], BF16, tag=f"wq{ib}", name=f"wq{ib}")
                    nc.sync.dma_start(out=w_t[:, :], in_=wqt[ib * P:(ib + 1) * P, :])
                    wq_t.append(w_t)
                for sbi in range(2):
                    qin_t = []
                    for ib in range(8):
                        t = inp.tile([P, 512], BF16, tag=f"in{ib}", name=f"qin{ib}_{sbi}")
                        nc.sync.dma_start(
                            out=t[:, :],
                            in_=qt_in[ib * P:(ib + 1) * P, sbi * 512:(sbi + 1) * 512])
                        qin_t.append(t)
                    for j in range(NJ):
                        pq = prj.tile([P, 512], F32, tag="prj", name=f"pq{sbi}_{j}")
                        for ib in range(8):
                            nc.tensor.matmul(
                                pq[:, :],
                                lhsT=wq_t[ib][:, j * P:(j + 1) * P],
                                rhs=qin_t[ib][:, :],
                                start=(ib == 0), stop=(ib == 7))
                        qt_t = qtmpp.tile([P, 512], F32R, tag="qtmp", name=f"qt{sbi}_{j}")
                        nc.scalar.activation(qt_t[:, :], pq[:, :], AF.Identity,
                                             bias=bqt_sb[:, j:j + 1])
                        pqn = prn.tile([P, 512], F32, tag="prn", name=f"pqn{sbi}_{j}")
                        nc.tensor.matmul(pqn[:, :], lhsT=naqw_sb[:, :],
                                         rhs=qt_t[:, :], start=True, stop=True)
                        nc.scalar.activation(
                            qn[j][:, sbi * 512:(sbi + 1) * 512], pqn[:, :],
                            AF.Tanh, bias=naqb_sb[:, :])
                # fold 1/(sqrt(DK)*temp_h) into qn
                for j in range(NJ):
                    nc.vector.tensor_scalar_mul(qn[j][:, :], qn[j][:, :],
                                                tscp_sb[:, j:j + 1])

            esp.close()   # frees proj psum + shared input pool

            # ---------------- attention ------------------------------------
            ctx = []
            with tc.tile_pool(name="maskp", bufs=1) as maskp, \
                 tc.tile_pool(name="pp", bufs=4) as ppool, \
                 tc.tile_pool(name="pss", bufs=1, space="PSUM") as pss, \
                 tc.tile_pool(name="psc", bufs=2, space="PSUM") as psc:
                mask_t = []
                for kt in range(NKT):
                    t = maskp.tile([P, SQ], BF16, tag=f"m{kt}", name=f"mask{kt}")
                    nc.sync.dma_start(out=t[:, :],
                                      in_=maskt[kt * P:(kt + 1) * P, :])
                    mask_t.append(t)
                for j in range(NJ):
                    for qb in range(2):
                        ctx_ps = [psc.tile([DK + 1, 512], F32, tag="ctx_ps",
                                           name=f"ctxps{j}_{qb}_{h2}", bufs=2)
                                  for h2 in range(2)]
                        for kt in range(NKT):
                            ps_s = pss.tile([P, SQ], F32, tag="ps_s",
                                            name=f"pss{j}_{qb}_{kt}", bufs=3)
                            for h2 in range(2):
                                nc.tensor.matmul(
                                    ps_s[:, h2 * 512:(h2 + 1) * 512],
                                    lhsT=knt[j][h2 * DK:(h2 + 1) * DK, kt * P:(kt + 1) * P],
                                    rhs=qn[j][h2 * DK:(h2 + 1) * DK, qb * 512:(qb + 1) * 512],
                                    start=True, stop=True)
                            p_t = ppool.tile([P, SQ], BF16, tag="p",
                                             name=f"p{j}_{qb}_{kt}")
                            nc.scalar.activation(p_t[:, :], ps_s[:, :], AF.Exp)
                            nc.vector.tensor_tensor(
                                p_t[:, :].rearrange("p (a b) -> p a b", a=2),
                                p_t[:, :].rearrange("p (a b) -> p a b", a=2),
                                mask_t[kt][:, None, qb * 512:(qb + 1) * 512]
                                    .to_broadcast((P, 2, 512)),
                                op=OP.mult)
                            for h2 in range(2):
                                nc.tensor.matmul(
                                    ctx_ps[h2][:, :],
                                    lhsT=vaug[kt][:, 2 * j + h2, :],
                                    rhs=p_t[:, h2 * 512:(h2 + 1) * 512],
                                    start=(kt == 0), stop=(kt == NKT - 1))
                        if qb == 0:
                            ctx_j = qcp.tile([P, SQ], F32R, tag="qc", name=f"ctx{j}")
                            ctx.append(ctx_j)
                        for h2 in range(2):
                            h = 2 * j + h2
                            nc.vector.tensor_scalar(
                                ctx_j[h2 * DK:(h2 + 1) * DK, qb * 512:(qb + 1) * 512],
                                ctx_ps[h2][0:DK, :], 1.0, None, op0=OP.mult)
                            sstage = ppool.tile([1, 512], F32, tag="sstage",
                                                name=f"sst{j}_{qb}_{h2}", bufs=2)
                            nc.vector.tensor_scalar(sstage[0:1, :],
                                                    ctx_ps[h2][DK:DK + 1, :],
                                                    1.0, None, op0=OP.mult)
                            nc.sync.dma_start(
                                out=sums_d[h:h + 1, qb * 512:(qb + 1) * 512],
                                in_=sstage[0:1, :])
                    for h2 in range(2):
                        h = 2 * j + h2
                        # reshape through DRAM so the reciprocal runs 128-wide
                        srow = ppool.tile([P, SQ // P], F32, tag="srow",
                                          name=f"srow{j}_{h2}", bufs=2)
                        nc.sync.dma_start(
                            out=srow[:, :],
                            in_=sums_d[h, :].rearrange("(p c) -> p c", p=P))
                        nc.vector.reciprocal(srow[:, :], srow[:, :])
                        nc.sync.dma_start(
                            out=recip_d[h, :].rearrange("(p c) -> p c", p=P),
                            in_=srow[:, :])
                    bc = ppool.tile([P, SQ], F32, tag="bc", name=f"bc{j}", bufs=2)
                    nc.sync.dma_start(
                        out=bc[0:DK, :],
                        in_=recip_d[2 * j:2 * j + 1, :].to_broadcast((DK, SQ)))
                    nc.sync.dma_start(
                        out=bc[DK:P, :],
                        in_=recip_d[2 * j + 1:2 * j + 2, :].to_broadcast((DK, SQ)))
                    nc.vector.scalar_tensor_tensor(
                        ctx_j[:, :], ctx_j[:, :], 1.0, bc[:, :],
                        op0=OP.mult, op1=OP.mult)

            if debug:
                nc.sync.dma_start(out=sumsd[:, :], in_=sums_d[:, :])
                for j in range(NJ):
                    nc.sync.dma_start(out=qnd[j * P:(j + 1) * P, :], in_=qn[j][:, :])
                    nc.sync.dma_start(out=knd[j * P:(j + 1) * P, :], in_=knt[j][:, :])
                    nc.sync.dma_start(out=ctxd[j * P:(j + 1) * P, :], in_=ctx[j][:, :])
                for st in range(NKT):
                    nc.sync.dma_start(
                        out=vd[st * P:(st + 1) * P, :],
                        in_=vaug[st][:, :, :].rearrange("p a b -> p (a b)"))

            # kn / v_aug no longer needed
            es2.close()

            if debug:
                for j in range(NJ):
                    nc.sync.dma_start(out=ctxrd[j * P:(j + 1) * P, :],
                                      in_=ctx[j][:, :].bitcast(F32))

            # ------------- out-proj + RMSNorm --------------------------
            if True:
                with tc.tile_pool(name="wop", bufs=1) as wop, \
                     tc.tile_pool(name="outp", bufs=2) as outp, \
                     tc.tile_pool(name="scrp", bufs=2) as scrp, \
                     tc.tile_pool(name="pop", bufs=6, space="PSUM") as pop:
                    wo_t = []
                    for ib in range(8):
                        w_t = wop.tile([P, D], F32R, tag=f"wo{ib}", name=f"wo{ib}")
                        nc.sync.dma_start(out=w_t[:, :], in_=wot[ib * P:(ib + 1) * P, :])
                        wo_t.append(w_t)
                    rmsw_sb = wrk.tile([P, D], F32, name="rmsw_sb")
                    nc.sync.dma_start(out=rmsw_sb[:, :], in_=rmsw[:, :])
                    eps_t = wrk.tile([P, 1], F32, name="eps_t")
                    nc.vector.memset(eps_t[:, :], EPS)
                    for st in range(8):
                        o_sb = outp.tile([P, D], F32, tag="o", name=f"o{st}")
                        for ob in range(2):
                            po = pop.tile([P, 512], F32, tag="po", name=f"po{st}_{ob}")
                            for db in range(8):
                                nc.tensor.matmul(
                                    po[:, :],
                                    lhsT=ctx[db][:, st * P:(st + 1) * P],
                                    rhs=wo_t[db][:, ob * 512:(ob + 1) * 512],
                                    start=(db == 0), stop=False)
                            nc.tensor.matmul(
                                po[:, :], lhsT=onesr_sb[:, :],
                                rhs=bor_sb[:, ob * 512:(ob + 1) * 512],
                                start=False, stop=True)
                            nc.vector.tensor_scalar(o_sb[:, ob * 512:(ob + 1) * 512],
                                                    po[:, :], 1.0, None, op0=OP.mult)
                        sq_t = scrp.tile([P, D], F32, tag="sq", name=f"sq{st}")
                        ssq = scrp.tile([P, 1], F32, tag="ssq", name=f"ssq{st}")
                        nc.vector.scalar_tensor_tensor(
                            sq_t[:, :], o_sb[:, :], 1.0, o_sb[:, :],
                            op0=OP.mult, op1=OP.mult, accum_out=ssq[:, :])
                        rms1 = scrp.tile([P, 1], F32, tag="rms1", name=f"rms1{st}")
                        nc.scalar.activation(rms1[:, :], ssq[:, :], AF.Sqrt,
                                             bias=eps_t[:, :], scale=1.0 / D)
                        nc.vector.reciprocal(rms1[:, :], rms1[:, :])
                        o_f = outp.tile([P, D], F32, tag="of", name=f"of{st}")
                        nc.vector.scalar_tensor_tensor(
                            o_f[:, :], o_sb[:, :], rms1[:, :], rmsw_sb[:, :],
                            op0=OP.mult, op1=OP.mult)
                        nc.sync.dma_start(out=out[st * P:(st + 1) * P, :],
                                          in_=o_f[:, :])

    nc.compile()
    return nc


_NC_CACHE = []


def _get_nc():
    if not _NC_CACHE:
        _NC_CACHE.append(build_nc())
    return _NC_CACHE[0]


def _blockdiag2(t):
    bd = np.zeros((P, P), np.float32)
    bd[0:DK, 0:DK] = t
    bd[DK:P, DK:P] = t
    return bd


def _prep_in_maps(Q, K, V, mask, wq, bq, wk, bk, wv, bv, wo, bo,
                  na_q_w, na_q_b, na_k_w, na_k_b, temperature, rms_w):
    f = lambda x: np.asarray(x, dtype=np.float32)
    Q, K, V = f(Q), f(K), f(V)
    mask = np.asarray(mask)

    shared = dict(
        wqt=np.ascontiguousarray(f(wq).T).astype(ml_dtypes.bfloat16),
        wkt=_f32r_round(f(wk).T),
        wvt=np.ascontiguousarray(f(wv).T).astype(ml_dtypes.bfloat16),
        wot=_f32r_round(f(wo).T),
        bqt=np.ascontiguousarray(f(bq).reshape(8, P).T),
        bkt=np.ascontiguousarray(f(bk).reshape(8, P).T),
        bvr=_f32r_round(f(bv).reshape(1, D)),
        bor=_f32r_round(f(bo).reshape(1, D)),
        naqw=_f32r_round(_blockdiag2(f(na_q_w).T)),
        nakw=_f32r_round(_blockdiag2(f(na_k_w).T)),
        naqb=np.ascontiguousarray(np.tile(f(na_q_b), 2).reshape(P, 1)),
        nakb=np.ascontiguousarray(np.tile(f(na_k_b), 2).reshape(P, 1)),
        rmsw=np.ascontiguousarray(np.broadcast_to(f(rms_w), (P, D))),
        onesr=np.ones((1, P), np.float32),
    )
    ts = 1.0 / (np.sqrt(DK).astype(np.float32) * f(temperature).reshape(H))
    tscp = np.empty((P, 8), np.float32)
    for j in range(NJ):
        tscp[0:DK, j] = ts[2 * j]
        tscp[DK:P, j] = ts[2 * j + 1]
    shared["tscp"] = tscp

    kts, vts = {}, {}
    for b in range(B):
        kts[b] = _f32r_round(K[b].T)
        vts[b] = np.ascontiguousarray(V[b].T).astype(ml_dtypes.bfloat16)

    in_maps = []
    for c in range(NCORES):
        b, hf = divmod(c, 2)
        qsl = slice(hf * SQ, (hf + 1) * SQ)
        m = dict(shared)
        m["qt_in"] = np.ascontiguousarray(Q[b, qsl, :].T).astype(ml_dtypes.bfloat16)
        m["kt_in"] = kts[b]
        m["vt_in"] = vts[b]
        m["maskt"] = np.ascontiguousarray(
            mask[b, 0, qsl, :].T).astype(ml_dtypes.bfloat16)
        in_maps.append(m)
    return in_maps


def _run(in_maps, **kwargs):
    nc = _get_nc()
    return run_bass_kernel_spmd(nc, in_maps, core_ids=list(range(NCORES)), **kwargs)


def kernel(**inputs):
    in_maps = _prep_in_maps(**inputs)
    res = _run(in_maps)
    out = np.empty((B, S, D), np.float32)
    for c in range(NCORES):
        b, hf = divmod(c, 2)
        out[b, hf * SQ:(hf + 1) * SQ, :] = res.results[c]["out"]
    return out
